# revision 25
# baseline (speedup 1.0000x reference)
"""Trainium2 Bass kernel for nn_Pndb_43344809951805 (scatter_memory).

Data-parallel over batch B=16 across 8 NeuronCores (2 batches/core).

Algebraic rewrites vs the reference:
  Phase 1: scores = (questions @ Wk) @ raw^T  (q.bk bias is softmax-
           invariant over s). Wi is folded in as a 65th stationary
           column, so the v-gate logit row comes free with the scores
           matmul; sigma(g) rides the U transpose and scales the attn
           rows per-partition.
  Phase 2: one [65,512] matmul group per block (stationary = woq chunk
           plus a Wu1 column) yields the read logits transposed and the
           G1 gate row. boq enters as the exp activation's
           per-partition bias.
Scalar engine runs Exp/Copy only (sigmoids via exp to avoid activation
table reloads). DMA is split across both HWDGE rings (sync=SP,
scalar=ACT). HBM-bound: ~42 MB/core.
"""
import sys

sys.path.insert(0, "/opt/trn_rl_repo")

import numpy as np
import ml_dtypes

import concourse.bass as bass
import concourse.bacc as bacc
import concourse.mybir as mybir
import concourse.tile as tile
from concourse import masks
from concourse.bass_utils import run_bass_kernel_spmd

F32 = mybir.dt.float32
BF16 = mybir.dt.bfloat16
F8 = mybir.dt.float8e4
SCL = 64.0
AF = mybir.ActivationFunctionType
ALU = mybir.AluOpType
BF = ml_dtypes.bfloat16
F8NP = ml_dtypes.float8_e4m3fn
DR = mybir.MatmulPerfMode.DoubleRow

B, S, D, Q = 16, 2048, 1024, 64
NCORES = 8
BL = B // NCORES          # local batches per core
SBLK = 512                # s-block
NSB = S // SBLK           # 4 s-blocks per batch
NCH = S // 128            # 16 s-chunks per batch
NJ = D // 128             # 8 contraction chunks
CPB = SBLK // 128         # 4 chunks per s-block
QX = Q + 1                # extra fused gate column/row
QXP = 128                 # padded stationary width: dual-fp8 LDWEIGHTS
                          # requires all 4 PE column groups active, so the
                          # stationary must span 128 columns (65.. are 0)

_prog_cache = {}


def _build(bi_v: float, cgate_v: float):
    nc = bacc.Bacc("TRN2", target_bir_lowering=False, debug=False,
                   enable_asserts=False, num_devices=NCORES)

    rawT_d = nc.dram_tensor("rawT", [BL, NJ, 128, S], F8,
                            kind="ExternalInput")
    rawN_d = nc.dram_tensor("rawN", [BL * NCH, 128, D], F8,
                            kind="ExternalInput")
    pdT_d = nc.dram_tensor("pdT", [BL, NJ, 128, S], F8,
                           kind="ExternalInput")
    pdN_d = nc.dram_tensor("pdN", [BL * NCH, 128, D], BF16,
                           kind="ExternalInput")
    qkx_d = nc.dram_tensor("qkx", [NJ, 128, QXP], F8, kind="ExternalInput")
    wox_d = nc.dram_tensor("wox", [NJ, 128, QXP], F8, kind="ExternalInput")
    boq_d = nc.dram_tensor("boq", [Q, 1], F32, kind="ExternalInput")
    wu2B_d = nc.dram_tensor("wu2B", [Q, D], F32, kind="ExternalInput")
    out_d = nc.dram_tensor("out", [BL * NCH, 128, D], BF16,
                           kind="ExternalOutput")

    with tile.TileContext(nc) as tc:
        with (
            tc.tile_pool(name="const", bufs=1) as cp,
            tc.tile_pool(name="dram", bufs=1, space="DRAM") as dram,
        ):
            # warm-up collective, dependency-free: the CC stream's
            # ~90us cold start (8-core barrier + first-collective setup)
            # runs during phase 1; contents are garbage and never read
            ar_w = dram.tile([1, 16], BF16)
            ar_wo = dram.tile([1, 16], BF16)
            nc.gpsimd.collective_compute(
                "AllReduce", ALU.add,
                replica_groups=[list(range(NCORES))],
                ins=[ar_w.opt()], outs=[ar_wo.opt()],
            )
            ident = cp.tile([128, 128], BF16, tag="ident")
            masks.make_identity(nc, ident[:])
            nbiB = cp.tile([128, 1], F32, tag="nbiB")
            nc.vector.memset(nbiB[:], -bi_v)
            ncgB = cp.tile([128, 1], F32, tag="ncgB")
            nc.vector.memset(ncgB[:], -cgate_v)

            qkx = cp.tile([128, NJ * QXP], F8, tag="qkx")
            wox = cp.tile([128, NJ * QXP], F8, tag="wox")
            boqc = cp.tile([Q, 1], F32, tag="boqc")
            wu2B = cp.tile([Q, D], F32, tag="wu2B")

            A_acc = cp.tile([Q, D], F32, tag="A_acc")
            A_bf = cp.tile([Q, D], BF16, tag="A_bf")
            awB = cp.tile([128, Q], BF16, tag="awB")
            scrA = cp.tile([Q, D], F32, tag="scrA")
            aw = cp.tile([Q, 1], F32, tag="aw")
            ar_in0 = dram.tile([Q + 1, 512 + 8], BF16)
            ar_out0 = dram.tile([Q + 1, 512 + 8], BF16)
            ar_in1 = dram.tile([Q, 512], BF16)
            ar_out1 = dram.tile([Q, 512], BF16)
            awz = cp.tile([Q, 8], BF16, tag="awz")
            nc.vector.memset(awz[:], 0.0)
            zrow = cp.tile([1, 512 + 8], BF16, tag="zrow")
            nc.vector.memset(zrow[:], 0.0)
            nc.gpsimd.dma_start(ar_in0[0:Q, 512:512 + 8], awz[:])
            nc.gpsimd.dma_start(ar_in0[Q:Q + 1, Q:512 + 8],
                                zrow[:, Q:512 + 8])

            # phase-2 pdT (both batches) + first pdN blocks prefetched
            # late in phase 1
            pdt0 = cp.tile([128, NJ * S], F8, tag="pdt0")
            pdt1 = cp.tile([128, NJ * S], F8, tag="pdt1")
            pdn_pre = [cp.tile([128, CPB * D], BF16, tag=f"pdnpre{k}",
                               name=f"pdnpre{k}")
                       for k in range(2)]

            def load_batchT(tile_, dram_t, b, s0=0, s1=S):
                nc.sync.dma_start(
                    tile_[:, :].rearrange("p (j s) -> p j s", j=NJ)
                    [:, :, s0:s1],
                    dram_t[b].rearrange("j p s -> p j s")[:, :, s0:s1])

            def load_n(tile_, dram_t, b, sb, eng):
                c0 = b * NCH + sb * CPB
                eng.dma_start(
                    tile_[:].rearrange("p (c d) -> p c d", c=CPB),
                    dram_t[c0:c0 + CPB].rearrange("c p d -> p c d"))

            # ================= PHASE 1 =================
            with (
                tc.tile_pool(name="p1", bufs=1) as p1,
                tc.tile_pool(name="p1ps", bufs=1, space="PSUM") as p1ps,
            ):
                def load_rn(b, sb):
                    t = p1.tile([128, CPB * D], F8, tag="rn",
                                name="rn", bufs=2)
                    load_n(t, rawN_d, b, sb, nc.scalar)
                    return t

                # weights first (tiny, needed by first matmuls)
                nc.sync.dma_start(
                    qkx[:].rearrange("p (j c) -> p j c", j=NJ),
                    qkx_d.rearrange("j p c -> p j c"))
                rawt = p1.tile([128, NJ * S], F8, tag="rawt0")
                load_batchT(rawt, rawT_d, 0, 0, SBLK)
                rn_cur = load_rn(0, 0)
                load_batchT(rawt, rawT_d, 0, SBLK, S)
                nc.sync.dma_start(
                    wox[:].rearrange("p (j c) -> p j c", j=NJ),
                    wox_d.rearrange("j p c -> p j c"))
                nc.sync.dma_start(boqc[:], boq_d[:])
                nc.sync.dma_start(wu2B[:], wu2B_d[:])

                for b in range(BL):
                    Zp = p1.tile([Q, NSB], F32, tag="Zp", bufs=2)
                    A_ps = p1ps.tile([128, D], F32, tag="A_ps", bufs=2)
                    for sb in range(NSB):
                        # prefetch next block's data
                        if sb + 1 < NSB:
                            rn_nxt = load_rn(b, sb + 1)
                        elif b + 1 < BL:
                            rn_nxt = load_rn(b + 1, 0)
                        else:
                            rn_nxt = None
                        if b == 0 and sb == 0:
                            rawt_nxt = p1.tile([128, NJ * S], F8,
                                               tag="rawt1")
                            load_batchT(rawt_nxt, rawT_d, 1)
                        if b == 1 and sb == 1:
                            load_batchT(pdt0, pdT_d, 0)
                            load_n(pdn_pre[0], pdN_d, 0, 0, nc.scalar)
                            load_n(pdn_pre[1], pdN_d, 0, 1, nc.scalar)
                        if b == 1 and sb == 2:
                            load_batchT(pdt1, pdT_d, 1)

                        # scores U[0:64] = exp(qk @ raw^T);
                        # row 64 = exp(-(raw.Wi + bi)) for the v-gate
                        sc_ps = p1ps.tile([QXP, SBLK], F32, tag="sc_ps",
                                          bufs=2)
                        qkx3 = qkx[:].rearrange("p (j c) -> p j c", j=NJ)
                        rawt3 = rawt[:].rearrange("p (j s) -> p j s", j=NJ)
                        for j in range(0, NJ, 2):
                            nc.tensor.matmul(
                                sc_ps[:], qkx3[:, j:j + 2, :],
                                rawt3[:, j:j + 2, sb * SBLK:
                                      (sb + 1) * SBLK],
                                start=(j == 0), stop=(j == NJ - 2),
                                perf_mode=DR)
                        U = p1.tile([QX, SBLK], BF16, tag="U", bufs=2)
                        nc.scalar.activation(U[0:Q, :], sc_ps[0:Q, :],
                                             AF.Exp, scale=1.0 / SCL,
                                             accum_out=Zp[:, sb:sb + 1])
                        nc.scalar.activation(U[Q:QX, :], sc_ps[Q:QX, :],
                                             AF.Exp, scale=-1.0 / SCL,
                                             bias=nbiB[0:1, :])
                        # transpose U chunks; fold g in on the way out
                        utp = None
                        rn3 = rn_cur[:].rearrange("p (c d) -> p c d",
                                                  c=CPB)
                        for cc in range(CPB):
                            ut_ps = p1ps.tile([128, QX], BF16, tag="ut_ps",
                                              bufs=2)
                            nc.tensor.transpose(
                                ut_ps[:], U[:, cc * 128:(cc + 1) * 128],
                                ident[:QX, :QX])
                            gcol = p1.tile([128, 1], F32, tag="gcol",
                                           bufs=4)
                            nc.vector.tensor_scalar_add(
                                gcol[:], ut_ps[:, Q:QX], 1.0)
                            nc.vector.reciprocal(gcol[:], gcol[:])
                            if cc % 2 == 0:
                                utp = p1.tile([128, 2 * 128], F8, tag="utp",
                                              name="utp", bufs=4)
                            nc.vector.tensor_scalar_mul(
                                utp[:, (cc % 2) * 128:
                                    (cc % 2) * 128 + Q],
                                ut_ps[:, 0:Q], gcol[:])
                            if cc % 2 == 1:
                                pr = sb * 2 + cc // 2
                                utp3 = utp[:].rearrange(
                                    "p (k m) -> p k m", k=2)
                                for h in range(2):
                                    nc.tensor.matmul(
                                        A_ps[:128, h * 512:(h + 1) * 512],
                                        utp3[:],
                                        rn3[:, cc - 1:cc + 1,
                                            h * 512:(h + 1) * 512],
                                        start=(pr == 0), stop=(pr == 7),
                                        skip_group_check=True,
                                        perf_mode=DR)
                        rn_cur = rn_nxt

                    # A_acc += A_ps / (16 * Z)
                    Z1 = p1.tile([Q, 1], F32, tag="Z1", bufs=2)
                    nc.vector.tensor_reduce(Z1[:], Zp[:], mybir.AxisListType.X,
                                            ALU.add)
                    sA = p1.tile([Q, 1], F32, tag="sA", bufs=2)
                    nc.vector.reciprocal(sA[:], Z1[:])
                    nc.vector.tensor_scalar_mul(sA[:], sA[:], 1.0 / B)
                    if b == 0:
                        nc.vector.tensor_scalar_mul(A_acc[:], A_ps[0:Q, :],
                                                    sA[:])
                        rawt = rawt_nxt
                    else:
                        nc.vector.scalar_tensor_tensor(
                            A_acc[:], A_ps[0:Q, :], sA[:], A_acc[:],
                            ALU.mult, ALU.add)

                # aw_local = A_acc . Wu2 rides along in AR0
                nc.vector.scalar_tensor_tensor(
                    scrA[:], A_acc[:], 1.0, wu2B[:],
                    ALU.mult, ALU.mult, accum_out=aw[:])
                nc.gpsimd.dma_start(ar_in0[0:Q, 0:512], A_acc[:, 0:512])
                nc.gpsimd.dma_start(
                    ar_in0[Q:Q + 1, 0:Q].rearrange("a b -> b a"), aw[:])
                nc.gpsimd.dma_start(ar_in1[0:Q, 0:512], A_acc[:, 512:D])

            # ---- AllReduce of partial A (split by D-half; gates and the
            # h0 a2 matmuls only need AR0) ----
            nc.gpsimd.collective_compute(
                "AllReduce", ALU.add,
                replica_groups=[list(range(NCORES))],
                ins=[ar_in0.opt()], outs=[ar_out0.opt()],
            )
            nc.gpsimd.collective_compute(
                "AllReduce", ALU.add,
                replica_groups=[list(range(NCORES))],
                ins=[ar_in1.opt()], outs=[ar_out1.opt()],
            )
            nc.gpsimd.dma_start(A_bf[:, 0:512], ar_out0[0:Q, 0:512])
            nc.gpsimd.dma_start(
                awB[:], ar_out0[Q:Q + 1, 0:Q].broadcast_to([128, Q]))
            nc.gpsimd.dma_start(A_bf[:, 512:D], ar_out1[0:Q, 0:512])

            # ================= PHASE 2 =================
            with (
                tc.tile_pool(name="p2", bufs=1) as p2,
                tc.tile_pool(name="p2ps", bufs=1, space="PSUM") as p2ps,
            ):
                def emit_partB(st):
                    (b, sb, Z2, G1, ut2x, u2s, pdn, SC) = st
                    G2 = p2.tile([128, CPB], F32, tag="G2", bufs=2)
                    for cc in range(CPB):
                        scr2 = p2.tile([128, Q], BF16, tag="scr2", bufs=2)
                        nc.vector.scalar_tensor_tensor(
                            scr2[:], u2s[cc][:], 1.0, awB[:],
                            ALU.mult, ALU.mult,
                            accum_out=G2[:, cc:cc + 1])
                    # SC = sigmoid(G1 + G2/Z2 + cg) / Z2  (exp-form)
                    rz = p2.tile([128, CPB], F32, tag="rz", bufs=2)
                    nc.vector.reciprocal(rz[:], Z2[:])
                    t4 = p2.tile([128, CPB], F32, tag="t4", bufs=2)
                    nc.vector.tensor_mul(t4[:], G2[:], rz[:])
                    nc.vector.tensor_add(t4[:], t4[:], G1[:])
                    e4 = p2.tile([128, CPB], F32, tag="e4", bufs=2)
                    nc.scalar.activation(e4[:], t4[:], AF.Exp,
                                         scale=-1.0, bias=ncgB[:])
                    nc.vector.tensor_scalar_add(e4[:], e4[:], 1.0)
                    nc.vector.reciprocal(e4[:], e4[:])
                    nc.vector.tensor_mul(SC[:], e4[:], rz[:])
                    emit_h(st, 0)

                def emit_h(st, h):
                    (b, sb, Z2, G1, ut2x, u2s, pdn, SC) = st
                    oh = p2.tile([128, CPB * 512], BF16, tag=f"oh{h}",
                                 name=f"oh{h}", bufs=3)
                    for cc in range(CPB):
                        a2_ps = p2ps.tile([128, 512], F32, tag="a2_ps",
                                          bufs=3)
                        nc.tensor.matmul(
                            a2_ps[:],
                            ut2x[0:Q, cc * 128:(cc + 1) * 128],
                            A_bf[:, h * 512:(h + 1) * 512],
                            start=True, stop=True)
                        osl = slice(cc * 512, (cc + 1) * 512)
                        psl = slice(cc * D + h * 512, cc * D + (h + 1) * 512)
                        if h == 0:
                            nc.scalar.activation(oh[:, osl], a2_ps[:],
                                                 AF.Copy,
                                                 scale=SC[:, cc:cc + 1])
                            nc.vector.tensor_add(
                                oh[:, osl], oh[:, osl], pdn[:, psl])
                        else:
                            nc.vector.scalar_tensor_tensor(
                                oh[:, osl], a2_ps[:], SC[:, cc:cc + 1],
                                pdn[:, psl], ALU.mult, ALU.add)
                    c0 = b * NCH + sb * CPB
                    nc.sync.dma_start(
                        out_d[c0:c0 + CPB, :, h * 512:(h + 1) * 512]
                        .rearrange("c p d -> p c d"),
                        oh[:].rearrange("p (c d) -> p c d", c=CPB))

                blocks = [(b, sb) for b in range(BL) for sb in range(NSB)]
                pdt_cur = pdt0
                pdn_queue = []
                pending = []
                for idx, (b, sb) in enumerate(blocks):
                    if b == 1 and sb == 0:
                        pdt_cur = pdt1
                    if idx + 2 < len(blocks):
                        nb, nsb2 = blocks[idx + 2]
                        pdn_n = p2.tile([128, CPB * D], BF16, tag="pdn",
                                        name="pdn", bufs=6)
                        load_n(pdn_n, pdN_d, nb, nsb2, nc.sync)
                        pdn_queue.append(pdn_n)
                    pdn = pdn_pre[idx] if idx < 2 else pdn_queue.pop(0)

                    # ---- partA: s2T + exp + transpose(+G1) ----
                    s2t_ps = p2ps.tile([QXP, SBLK], F32, tag="s2t_ps",
                                       bufs=2)
                    wox3 = wox[:].rearrange("p (j c) -> p j c", j=NJ)
                    pdt3 = pdt_cur[:].rearrange("p (j s) -> p j s", j=NJ)
                    for j in range(0, NJ, 2):
                        nc.tensor.matmul(
                            s2t_ps[:], wox3[:, j:j + 2, :],
                            pdt3[:, j:j + 2,
                                 sb * SBLK:(sb + 1) * SBLK],
                            start=(j == 0), stop=(j == NJ - 2),
                            perf_mode=DR)
                    ut2x = p2.tile([QX, SBLK], BF16, tag="ut2x", bufs=8)
                    nc.scalar.activation(ut2x[0:Q, :], s2t_ps[0:Q, :],
                                         AF.Exp, scale=1.0 / SCL,
                                         bias=boqc[:])
                    nc.scalar.activation(ut2x[Q:QX, :], s2t_ps[Q:QX, :],
                                         AF.Copy, scale=1.0 / SCL)
                    Z2 = p2.tile([128, CPB], F32, tag="Z2", bufs=8)
                    G1 = p2.tile([128, CPB], F32, tag="G1", bufs=8)
                    SC = p2.tile([128, CPB], F32, tag="SCp", name="SCp",
                                 bufs=8)
                    u2s = []
                    for cc in range(CPB):
                        u2c_ps = p2ps.tile([128, QX], BF16, tag="u2c_ps",
                                           bufs=2)
                        nc.tensor.transpose(
                            u2c_ps[:], ut2x[:, cc * 128:(cc + 1) * 128],
                            ident[:QX, :QX])
                        u2 = p2.tile([128, Q], BF16, tag=f"u2_{cc}",
                                     name=f"u2_{cc}", bufs=8)
                        nc.scalar.activation(u2[:], u2c_ps[:, 0:Q],
                                             AF.Copy,
                                             accum_out=Z2[:, cc:cc + 1])
                        nc.scalar.copy(G1[:, cc:cc + 1], u2c_ps[:, Q:QX])
                        u2s.append(u2)
                    pending.append((b, sb, Z2, G1, ut2x, u2s, pdn, SC))

                for st in pending:
                    emit_partB(st)
                for st in pending:
                    emit_h(st, 1)

    nc.compile()
    return nc


def _get_prog(bi_v, cgate_v):
    key = (round(bi_v, 9), round(cgate_v, 9))
    if key not in _prog_cache:
        _prog_cache[key] = _build(bi_v, cgate_v)
    return _prog_cache[key]


def kernel(raw, post_dec, mask, questions, Wk, bk, Wi, bi, Wo, bo,
           Wu1, bu1, Wu2, bu2, b1, _trace=False):
    raw = np.asarray(raw, dtype=np.float32)
    post_dec = np.asarray(post_dec, dtype=np.float32)
    questions = np.asarray(questions, dtype=np.float32)
    Wk = np.asarray(Wk, dtype=np.float32)
    Wo = np.asarray(Wo, dtype=np.float32)

    bi_v = float(np.asarray(bi).reshape(-1)[0])
    cgate_v = float(np.asarray(bu1).reshape(-1)[0]
                    + np.asarray(bu2).reshape(-1)[0]
                    + np.asarray(b1).reshape(-1)[0])
    nc = _get_prog(bi_v, cgate_v)

    inv_sqrt_d = np.float32(1.0 / np.sqrt(D))
    inv_sqrt_q = np.float32(1.0 / np.sqrt(Q))
    # stationaries with the fused gate column
    qkx = np.zeros((D, QXP), np.float32)
    qkx[:, 0:Q] = (questions @ Wk).T * inv_sqrt_d
    qkx[:, Q] = np.asarray(Wi, np.float32).reshape(D)
    wox = np.zeros((D, QXP), np.float32)
    wox[:, 0:Q] = (questions @ Wo).T * inv_sqrt_q
    wox[:, Q] = np.asarray(Wu1, np.float32).reshape(D)
    qkx = np.ascontiguousarray(qkx.reshape(NJ, 128, QXP) * SCL).astype(F8NP)
    wox = np.ascontiguousarray(wox.reshape(NJ, 128, QXP) * SCL).astype(F8NP)
    boq = np.ascontiguousarray(
        ((questions @ np.asarray(bo, np.float32)) * inv_sqrt_q
         ).reshape(Q, 1)).astype(np.float32)
    wu2B = np.ascontiguousarray(
        np.broadcast_to(np.asarray(Wu2, np.float32).reshape(1, D), (Q, D)))

    in_maps = []
    for r in range(NCORES):
        bs = slice(r * BL, (r + 1) * BL)
        rawT = np.ascontiguousarray(
            raw[bs].transpose(0, 2, 1)).astype(F8NP).reshape(
            BL, NJ, 128, S)
        rawN = np.ascontiguousarray(raw[bs]).astype(F8NP).reshape(
            BL * NCH, 128, D)
        pdT = np.ascontiguousarray(
            post_dec[bs].transpose(0, 2, 1)).astype(F8NP).reshape(
            BL, NJ, 128, S)
        pdN = np.ascontiguousarray(post_dec[bs]).astype(BF).reshape(
            BL * NCH, 128, D)
        in_maps.append({
            "rawT": rawT, "rawN": rawN, "pdT": pdT, "pdN": pdN,
            "qkx": qkx, "wox": wox, "boq": boq, "wu2B": wu2B,
        })

    res = run_bass_kernel_spmd(nc, in_maps, core_ids=list(range(NCORES)),
                               trace=_trace)
    out = np.concatenate(
        [res.results[r]["out"].astype(np.float32).reshape(BL, S, D)
         for r in range(NCORES)],
        axis=0)
    if _trace:
        kernel._last_result = res
    return out


# revision 27
# speedup vs baseline: 1.0986x; 1.0986x over previous
"""Trainium2 Bass kernel for nn_Pndb_43344809951805 (scatter_memory).

Data-parallel over batch B=16 across 8 NeuronCores (2 batches/core).

Algebraic rewrites vs the reference:
  Phase 1: scores = (questions @ Wk) @ raw^T  (q.bk bias is softmax-
           invariant over s). Wi is folded in as a 65th stationary
           column, so the v-gate logit row comes free with the scores
           matmul; sigma(g) rides the U transpose and scales the attn
           rows per-partition.
  Phase 2: one [65,512] matmul group per block (stationary = woq chunk
           plus a Wu1 column) yields the read logits transposed and the
           G1 gate row. boq enters as the exp activation's
           per-partition bias.
Scalar engine runs Exp/Copy only (sigmoids via exp to avoid activation
table reloads). DMA is split across both HWDGE rings (sync=SP,
scalar=ACT). HBM-bound: ~42 MB/core.
"""
import sys

sys.path.insert(0, "/opt/trn_rl_repo")

import numpy as np
import ml_dtypes

import concourse.bass as bass
import concourse.bacc as bacc
import concourse.mybir as mybir
import concourse.tile as tile
from concourse import masks
from concourse.bass_utils import run_bass_kernel_spmd

F32 = mybir.dt.float32
BF16 = mybir.dt.bfloat16
F8 = mybir.dt.float8e4
SCL = 64.0
AF = mybir.ActivationFunctionType
ALU = mybir.AluOpType
BF = ml_dtypes.bfloat16
F8NP = ml_dtypes.float8_e4m3fn
DR = mybir.MatmulPerfMode.DoubleRow

B, S, D, Q = 16, 2048, 1024, 64
NCORES = 8
BL = B // NCORES          # local batches per core
SBLK = 512                # s-block
NSB = S // SBLK           # 4 s-blocks per batch
NCH = S // 128            # 16 s-chunks per batch
NJ = D // 128             # 8 contraction chunks
CPB = SBLK // 128         # 4 chunks per s-block
QX = Q + 1                # extra fused gate column/row
QXP = 128                 # padded stationary width: dual-fp8 LDWEIGHTS
                          # requires all 4 PE column groups active, so the
                          # stationary must span 128 columns (65.. are 0)

_prog_cache = {}


def _build(bi_v: float, cgate_v: float):
    nc = bacc.Bacc("TRN2", target_bir_lowering=False, debug=False,
                   enable_asserts=False, num_devices=NCORES)

    rawT_d = nc.dram_tensor("rawT", [BL, NJ, 128, S], F8,
                            kind="ExternalInput")
    rawN_d = nc.dram_tensor("rawN", [BL * NCH, 128, D], F8,
                            kind="ExternalInput")
    pdT_d = nc.dram_tensor("pdT", [BL, NJ, 128, S], F8,
                           kind="ExternalInput")
    pdN_d = nc.dram_tensor("pdN", [BL * NCH, 128, D], BF16,
                           kind="ExternalInput")
    qkx_d = nc.dram_tensor("qkx", [NJ, 128, QXP], F8, kind="ExternalInput")
    wox_d = nc.dram_tensor("wox", [NJ, 128, QXP], F8, kind="ExternalInput")
    boq_d = nc.dram_tensor("boq", [Q, 1], F32, kind="ExternalInput")
    wu2B_d = nc.dram_tensor("wu2B", [Q, D], F32, kind="ExternalInput")
    out_d = nc.dram_tensor("out", [BL * NCH, 128, D], BF16,
                           kind="ExternalOutput")

    with tile.TileContext(nc) as tc:
        with (
            tc.tile_pool(name="const", bufs=1) as cp,
            tc.tile_pool(name="dram", bufs=1, space="DRAM") as dram,
        ):
            # warm-up collective, dependency-free: the CC stream's
            # ~90us cold start (8-core barrier + first-collective setup)
            # runs during phase 1; contents are garbage and never read
            ar_w = dram.tile([1, 16], BF16)
            ar_wo = dram.tile([1, 16], BF16)
            nc.gpsimd.collective_compute(
                "AllReduce", ALU.add,
                replica_groups=[list(range(NCORES))],
                ins=[ar_w.opt()], outs=[ar_wo.opt()],
            )
            ident = cp.tile([128, 128], BF16, tag="ident")
            masks.make_identity(nc, ident[:])
            nbiB = cp.tile([128, 1], F32, tag="nbiB")
            nc.vector.memset(nbiB[:], -bi_v)
            ncgB = cp.tile([128, 1], F32, tag="ncgB")
            nc.vector.memset(ncgB[:], -cgate_v)

            qkx = cp.tile([128, NJ * QXP], F8, tag="qkx")
            wox = cp.tile([128, NJ * QXP], F8, tag="wox")
            boqc = cp.tile([Q, 1], F32, tag="boqc")
            wu2B = cp.tile([Q, D], F32, tag="wu2B")

            A_acc = cp.tile([Q, D], F32, tag="A_acc")
            A_bf = cp.tile([Q, D], BF16, tag="A_bf")
            awB = cp.tile([128, Q], BF16, tag="awB")
            scrA = cp.tile([Q, D], F32, tag="scrA")
            aw = cp.tile([Q, 1], F32, tag="aw")
            ar_in0 = dram.tile([Q + 1, 512 + 8], BF16)
            ar_out0 = dram.tile([Q + 1, 512 + 8], BF16)
            ar_in1 = dram.tile([Q, 512], BF16)
            ar_out1 = dram.tile([Q, 512], BF16)
            awz = cp.tile([Q, 8], BF16, tag="awz")
            nc.vector.memset(awz[:], 0.0)
            zrow = cp.tile([1, 512 + 8], BF16, tag="zrow")
            nc.vector.memset(zrow[:], 0.0)
            nc.gpsimd.dma_start(ar_in0[0:Q, 512:512 + 8], awz[:])
            nc.gpsimd.dma_start(ar_in0[Q:Q + 1, Q:512 + 8],
                                zrow[:, Q:512 + 8])

            # phase-2 pdT (both batches) + first pdN blocks prefetched
            # late in phase 1
            pdt0 = cp.tile([128, NJ * S], F8, tag="pdt0")
            pdt1 = cp.tile([128, NJ * S], F8, tag="pdt1")
            pdn_pre = [cp.tile([128, CPB * D], BF16, tag=f"pdnpre{k}",
                               name=f"pdnpre{k}")
                       for k in range(2)]

            def load_batchT(tile_, dram_t, b, s0=0, s1=S):
                nc.sync.dma_start(
                    tile_[:, :].rearrange("p (j s) -> p j s", j=NJ)
                    [:, :, s0:s1],
                    dram_t[b].rearrange("j p s -> p j s")[:, :, s0:s1])

            def load_n(tile_, dram_t, b, sb, eng):
                c0 = b * NCH + sb * CPB
                eng.dma_start(
                    tile_[:].rearrange("p (c d) -> p c d", c=CPB),
                    dram_t[c0:c0 + CPB].rearrange("c p d -> p c d"))

            # ================= PHASE 1 =================
            with (
                tc.tile_pool(name="p1", bufs=1) as p1,
                tc.tile_pool(name="p1ps", bufs=1, space="PSUM") as p1ps,
            ):
                def load_rn(b, sb):
                    t = p1.tile([128, CPB * D], F8, tag="rn",
                                name="rn", bufs=2)
                    load_n(t, rawN_d, b, sb, nc.scalar)
                    return t

                # weights first (tiny, needed by first matmuls)
                nc.sync.dma_start(
                    qkx[:].rearrange("p (j c) -> p j c", j=NJ),
                    qkx_d.rearrange("j p c -> p j c"))
                rawt = p1.tile([128, NJ * S], F8, tag="rawt0")
                load_batchT(rawt, rawT_d, 0, 0, SBLK)
                rn_cur = load_rn(0, 0)
                load_batchT(rawt, rawT_d, 0, SBLK, S)
                nc.sync.dma_start(
                    wox[:].rearrange("p (j c) -> p j c", j=NJ),
                    wox_d.rearrange("j p c -> p j c"))
                nc.sync.dma_start(boqc[:], boq_d[:])
                nc.sync.dma_start(wu2B[:], wu2B_d[:])

                for b in range(BL):
                    Zp = p1.tile([Q, NSB], F32, tag="Zp", bufs=2)
                    A_ps = p1ps.tile([128, D], F32, tag="A_ps", bufs=2)
                    for sb in range(NSB):
                        # prefetch next block's data
                        if sb + 1 < NSB:
                            rn_nxt = load_rn(b, sb + 1)
                        elif b + 1 < BL:
                            rn_nxt = load_rn(b + 1, 0)
                        else:
                            rn_nxt = None
                        if b == 0 and sb == 0:
                            rawt_nxt = p1.tile([128, NJ * S], F8,
                                               tag="rawt1")
                            load_batchT(rawt_nxt, rawT_d, 1)
                        if b == 1 and sb == 1:
                            load_batchT(pdt0, pdT_d, 0)
                            load_n(pdn_pre[0], pdN_d, 0, 0, nc.scalar)
                            load_n(pdn_pre[1], pdN_d, 0, 1, nc.scalar)
                        if b == 1 and sb == 2:
                            load_batchT(pdt1, pdT_d, 1)

                        # scores U[0:64] = exp(qk @ raw^T);
                        # row 64 = exp(-(raw.Wi + bi)) for the v-gate
                        sc_ps = p1ps.tile([QXP, SBLK], F32, tag="sc_ps",
                                          bufs=2)
                        qkx3 = qkx[:].rearrange("p (j c) -> p j c", j=NJ)
                        rawt3 = rawt[:].rearrange("p (j s) -> p j s", j=NJ)
                        for j in range(0, NJ, 2):
                            nc.tensor.matmul(
                                sc_ps[:], qkx3[:, j:j + 2, :],
                                rawt3[:, j:j + 2, sb * SBLK:
                                      (sb + 1) * SBLK],
                                start=(j == 0), stop=(j == NJ - 2),
                                perf_mode=DR)
                        U = p1.tile([QX, SBLK], BF16, tag="U", bufs=2)
                        nc.scalar.activation(U[0:Q, :], sc_ps[0:Q, :],
                                             AF.Exp, scale=1.0 / SCL,
                                             accum_out=Zp[:, sb:sb + 1])
                        nc.scalar.activation(U[Q:QX, :], sc_ps[Q:QX, :],
                                             AF.Exp, scale=-1.0 / SCL,
                                             bias=nbiB[0:1, :])
                        # transpose U chunks; fold g in on the way out
                        utp = None
                        rn3 = rn_cur[:].rearrange("p (c d) -> p c d",
                                                  c=CPB)
                        for cc in range(CPB):
                            ut_ps = p1ps.tile([128, QX], BF16, tag="ut_ps",
                                              bufs=2)
                            nc.tensor.transpose(
                                ut_ps[:], U[:, cc * 128:(cc + 1) * 128],
                                ident[:QX, :QX])
                            gcol = p1.tile([128, 1], F32, tag="gcol",
                                           bufs=4)
                            nc.vector.tensor_scalar_add(
                                gcol[:], ut_ps[:, Q:QX], 1.0)
                            nc.vector.reciprocal(gcol[:], gcol[:])
                            if cc % 2 == 0:
                                utp = p1.tile([128, 2 * 128], F8, tag="utp",
                                              name="utp", bufs=4)
                            nc.vector.tensor_scalar_mul(
                                utp[:, (cc % 2) * 128:
                                    (cc % 2) * 128 + Q],
                                ut_ps[:, 0:Q], gcol[:])
                            if cc % 2 == 1:
                                pr = sb * 2 + cc // 2
                                utp3 = utp[:].rearrange(
                                    "p (k m) -> p k m", k=2)
                                for h in range(2):
                                    nc.tensor.matmul(
                                        A_ps[:128, h * 512:(h + 1) * 512],
                                        utp3[:],
                                        rn3[:, cc - 1:cc + 1,
                                            h * 512:(h + 1) * 512],
                                        start=(pr == 0), stop=(pr == 7),
                                        skip_group_check=True,
                                        perf_mode=DR)
                        rn_cur = rn_nxt

                    # A_acc += A_ps / (16 * Z)
                    Z1 = p1.tile([Q, 1], F32, tag="Z1", bufs=2)
                    nc.vector.tensor_reduce(Z1[:], Zp[:], mybir.AxisListType.X,
                                            ALU.add)
                    sA = p1.tile([Q, 1], F32, tag="sA", bufs=2)
                    nc.vector.reciprocal(sA[:], Z1[:])
                    nc.vector.tensor_scalar_mul(sA[:], sA[:], 1.0 / B)
                    if b == 0:
                        nc.vector.tensor_scalar_mul(A_acc[:], A_ps[0:Q, :],
                                                    sA[:])
                        rawt = rawt_nxt
                    else:
                        nc.vector.scalar_tensor_tensor(
                            A_acc[:], A_ps[0:Q, :], sA[:], A_acc[:],
                            ALU.mult, ALU.add)

                # aw_local = A_acc . Wu2 rides along in AR0
                nc.vector.scalar_tensor_tensor(
                    scrA[:], A_acc[:], 1.0, wu2B[:],
                    ALU.mult, ALU.mult, accum_out=aw[:])
                nc.gpsimd.dma_start(ar_in0[0:Q, 0:512], A_acc[:, 0:512])
                nc.gpsimd.dma_start(
                    ar_in0[Q:Q + 1, 0:Q].rearrange("a b -> b a"), aw[:])
                nc.gpsimd.dma_start(ar_in1[0:Q, 0:512], A_acc[:, 512:D])

            # ---- AllReduce of partial A (split by D-half; gates and the
            # h0 a2 matmuls only need AR0) ----
            nc.gpsimd.collective_compute(
                "AllReduce", ALU.add,
                replica_groups=[list(range(NCORES))],
                ins=[ar_in0.opt()], outs=[ar_out0.opt()],
            )
            nc.gpsimd.collective_compute(
                "AllReduce", ALU.add,
                replica_groups=[list(range(NCORES))],
                ins=[ar_in1.opt()], outs=[ar_out1.opt()],
            )
            nc.gpsimd.dma_start(A_bf[:, 0:512], ar_out0[0:Q, 0:512])
            nc.gpsimd.dma_start(
                awB[:], ar_out0[Q:Q + 1, 0:Q].broadcast_to([128, Q]))
            nc.gpsimd.dma_start(A_bf[:, 512:D], ar_out1[0:Q, 0:512])

            # ================= PHASE 2 =================
            with (
                tc.tile_pool(name="p2", bufs=1) as p2,
                tc.tile_pool(name="p2ps", bufs=1, space="PSUM") as p2ps,
            ):
                def emit_partB(st):
                    (b, sb, Z2, G1, ut2x, u2s, pdn, SC) = st
                    G2 = p2.tile([128, CPB], F32, tag="G2", bufs=2)
                    for cc in range(CPB):
                        scr2 = p2.tile([128, Q], BF16, tag="scr2", bufs=2)
                        nc.vector.scalar_tensor_tensor(
                            scr2[:], u2s[cc][:], 1.0, awB[:],
                            ALU.mult, ALU.mult,
                            accum_out=G2[:, cc:cc + 1])
                    # SC = sigmoid(G1 + G2/Z2 + cg) / Z2  (exp-form)
                    rz = p2.tile([128, CPB], F32, tag="rz", bufs=2)
                    nc.vector.reciprocal(rz[:], Z2[:])
                    t4 = p2.tile([128, CPB], F32, tag="t4", bufs=2)
                    nc.vector.tensor_mul(t4[:], G2[:], rz[:])
                    nc.vector.tensor_add(t4[:], t4[:], G1[:])
                    e4 = p2.tile([128, CPB], F32, tag="e4", bufs=2)
                    nc.scalar.activation(e4[:], t4[:], AF.Exp,
                                         scale=-1.0, bias=ncgB[:])
                    nc.vector.tensor_scalar_add(e4[:], e4[:], 1.0)
                    nc.vector.reciprocal(e4[:], e4[:])
                    nc.vector.tensor_mul(SC[:], e4[:], rz[:])
                    emit_h(st, 0)

                def emit_h(st, h):
                    (b, sb, Z2, G1, ut2x, u2s, pdn, SC) = st
                    oh = p2.tile([128, CPB * 512], BF16, tag=f"oh{h}",
                                 name=f"oh{h}", bufs=3)
                    for cc in range(CPB):
                        a2_ps = p2ps.tile([128, 512], F32, tag="a2_ps",
                                          bufs=3)
                        nc.tensor.matmul(
                            a2_ps[:],
                            ut2x[0:Q, cc * 128:(cc + 1) * 128],
                            A_bf[:, h * 512:(h + 1) * 512],
                            start=True, stop=True)
                        osl = slice(cc * 512, (cc + 1) * 512)
                        psl = slice(cc * D + h * 512, cc * D + (h + 1) * 512)
                        if h == 0:
                            nc.scalar.activation(oh[:, osl], a2_ps[:],
                                                 AF.Copy,
                                                 scale=SC[:, cc:cc + 1])
                            nc.vector.tensor_add(
                                oh[:, osl], oh[:, osl], pdn[:, psl])
                        elif cc < 2:
                            nc.vector.scalar_tensor_tensor(
                                oh[:, osl], a2_ps[:], SC[:, cc:cc + 1],
                                pdn[:, psl], ALU.mult, ALU.add)
                        else:
                            nc.scalar.activation(oh[:, osl], a2_ps[:],
                                                 AF.Copy,
                                                 scale=SC[:, cc:cc + 1])
                            nc.vector.tensor_add(
                                oh[:, osl], oh[:, osl], pdn[:, psl])
                    c0 = b * NCH + sb * CPB
                    nc.sync.dma_start(
                        out_d[c0:c0 + CPB, :, h * 512:(h + 1) * 512]
                        .rearrange("c p d -> p c d"),
                        oh[:].rearrange("p (c d) -> p c d", c=CPB))

                blocks = [(b, sb) for b in range(BL) for sb in range(NSB)]
                pdt_cur = pdt0
                pdn_queue = []
                pending = []
                for idx, (b, sb) in enumerate(blocks):
                    if b == 1 and sb == 0:
                        pdt_cur = pdt1
                    if idx + 2 < len(blocks):
                        nb, nsb2 = blocks[idx + 2]
                        pdn_n = p2.tile([128, CPB * D], BF16, tag="pdn",
                                        name="pdn", bufs=6)
                        load_n(pdn_n, pdN_d, nb, nsb2, nc.sync)
                        pdn_queue.append(pdn_n)
                    pdn = pdn_pre[idx] if idx < 2 else pdn_queue.pop(0)

                    # ---- partA: s2T + exp + transpose(+G1) ----
                    s2t_ps = p2ps.tile([QXP, SBLK], F32, tag="s2t_ps",
                                       bufs=2)
                    wox3 = wox[:].rearrange("p (j c) -> p j c", j=NJ)
                    pdt3 = pdt_cur[:].rearrange("p (j s) -> p j s", j=NJ)
                    for j in range(0, NJ, 2):
                        nc.tensor.matmul(
                            s2t_ps[:], wox3[:, j:j + 2, :],
                            pdt3[:, j:j + 2,
                                 sb * SBLK:(sb + 1) * SBLK],
                            start=(j == 0), stop=(j == NJ - 2),
                            perf_mode=DR)
                    ut2x = p2.tile([QX, SBLK], BF16, tag="ut2x", bufs=8)
                    nc.scalar.activation(ut2x[0:Q, :], s2t_ps[0:Q, :],
                                         AF.Exp, scale=1.0 / SCL,
                                         bias=boqc[:])
                    nc.scalar.activation(ut2x[Q:QX, :], s2t_ps[Q:QX, :],
                                         AF.Copy, scale=1.0 / SCL)
                    Z2 = p2.tile([128, CPB], F32, tag="Z2", bufs=8)
                    G1 = p2.tile([128, CPB], F32, tag="G1", bufs=8)
                    SC = p2.tile([128, CPB], F32, tag="SCp", name="SCp",
                                 bufs=8)
                    u2s = []
                    for cc in range(CPB):
                        u2c_ps = p2ps.tile([128, QX], BF16, tag="u2c_ps",
                                           bufs=2)
                        nc.tensor.transpose(
                            u2c_ps[:], ut2x[:, cc * 128:(cc + 1) * 128],
                            ident[:QX, :QX])
                        u2 = p2.tile([128, Q], BF16, tag=f"u2_{cc}",
                                     name=f"u2_{cc}", bufs=8)
                        nc.scalar.activation(u2[:], u2c_ps[:, 0:Q],
                                             AF.Copy,
                                             accum_out=Z2[:, cc:cc + 1])
                        nc.scalar.copy(G1[:, cc:cc + 1], u2c_ps[:, Q:QX])
                        u2s.append(u2)
                    pending.append((b, sb, Z2, G1, ut2x, u2s, pdn, SC))

                for st in pending:
                    emit_partB(st)
                for st in pending:
                    emit_h(st, 1)

    nc.compile()
    return nc


def _get_prog(bi_v, cgate_v):
    key = (round(bi_v, 9), round(cgate_v, 9))
    if key not in _prog_cache:
        _prog_cache[key] = _build(bi_v, cgate_v)
    return _prog_cache[key]


def kernel(raw, post_dec, mask, questions, Wk, bk, Wi, bi, Wo, bo,
           Wu1, bu1, Wu2, bu2, b1, _trace=False):
    raw = np.asarray(raw, dtype=np.float32)
    post_dec = np.asarray(post_dec, dtype=np.float32)
    questions = np.asarray(questions, dtype=np.float32)
    Wk = np.asarray(Wk, dtype=np.float32)
    Wo = np.asarray(Wo, dtype=np.float32)

    bi_v = float(np.asarray(bi).reshape(-1)[0])
    cgate_v = float(np.asarray(bu1).reshape(-1)[0]
                    + np.asarray(bu2).reshape(-1)[0]
                    + np.asarray(b1).reshape(-1)[0])
    nc = _get_prog(bi_v, cgate_v)

    inv_sqrt_d = np.float32(1.0 / np.sqrt(D))
    inv_sqrt_q = np.float32(1.0 / np.sqrt(Q))
    # stationaries with the fused gate column
    qkx = np.zeros((D, QXP), np.float32)
    qkx[:, 0:Q] = (questions @ Wk).T * inv_sqrt_d
    qkx[:, Q] = np.asarray(Wi, np.float32).reshape(D)
    wox = np.zeros((D, QXP), np.float32)
    wox[:, 0:Q] = (questions @ Wo).T * inv_sqrt_q
    wox[:, Q] = np.asarray(Wu1, np.float32).reshape(D)
    qkx = np.ascontiguousarray(qkx.reshape(NJ, 128, QXP) * SCL).astype(F8NP)
    wox = np.ascontiguousarray(wox.reshape(NJ, 128, QXP) * SCL).astype(F8NP)
    boq = np.ascontiguousarray(
        ((questions @ np.asarray(bo, np.float32)) * inv_sqrt_q
         ).reshape(Q, 1)).astype(np.float32)
    wu2B = np.ascontiguousarray(
        np.broadcast_to(np.asarray(Wu2, np.float32).reshape(1, D), (Q, D)))

    in_maps = []
    for r in range(NCORES):
        bs = slice(r * BL, (r + 1) * BL)
        rawT = np.ascontiguousarray(
            raw[bs].transpose(0, 2, 1)).astype(F8NP).reshape(
            BL, NJ, 128, S)
        rawN = np.ascontiguousarray(raw[bs]).astype(F8NP).reshape(
            BL * NCH, 128, D)
        pdT = np.ascontiguousarray(
            post_dec[bs].transpose(0, 2, 1)).astype(F8NP).reshape(
            BL, NJ, 128, S)
        pdN = np.ascontiguousarray(post_dec[bs]).astype(BF).reshape(
            BL * NCH, 128, D)
        in_maps.append({
            "rawT": rawT, "rawN": rawN, "pdT": pdT, "pdN": pdN,
            "qkx": qkx, "wox": wox, "boq": boq, "wu2B": wu2B,
        })

    res = run_bass_kernel_spmd(nc, in_maps, core_ids=list(range(NCORES)),
                               trace=_trace)
    out = np.concatenate(
        [res.results[r]["out"].astype(np.float32).reshape(BL, S, D)
         for r in range(NCORES)],
        axis=0)
    if _trace:
        kernel._last_result = res
    return out


# revision 28
# speedup vs baseline: 1.2178x; 1.1085x over previous
"""Trainium2 Bass kernel for nn_Pndb_43344809951805 (scatter_memory).

Data-parallel over batch B=16 across 8 NeuronCores (2 batches/core).

Algebraic rewrites vs the reference:
  Phase 1: scores = (questions @ Wk) @ raw^T  (q.bk bias is softmax-
           invariant over s). Wi is folded in as a 65th stationary
           column, so the v-gate logit row comes free with the scores
           matmul; sigma(g) rides the U transpose and scales the attn
           rows per-partition.
  Phase 2: one [65,512] matmul group per block (stationary = woq chunk
           plus a Wu1 column) yields the read logits transposed and the
           G1 gate row. boq enters as the exp activation's
           per-partition bias.
Scalar engine runs Exp/Copy only (sigmoids via exp to avoid activation
table reloads). DMA is split across both HWDGE rings (sync=SP,
scalar=ACT). HBM-bound: ~42 MB/core.
"""
import sys

sys.path.insert(0, "/opt/trn_rl_repo")

import numpy as np
import ml_dtypes

import concourse.bass as bass
import concourse.bacc as bacc
import concourse.mybir as mybir
import concourse.tile as tile
from concourse import masks
from concourse.bass_utils import run_bass_kernel_spmd

F32 = mybir.dt.float32
BF16 = mybir.dt.bfloat16
F8 = mybir.dt.float8e4
SCL = 64.0
AF = mybir.ActivationFunctionType
ALU = mybir.AluOpType
BF = ml_dtypes.bfloat16
F8NP = ml_dtypes.float8_e4m3fn
DR = mybir.MatmulPerfMode.DoubleRow

B, S, D, Q = 16, 2048, 1024, 64
NCORES = 8
BL = B // NCORES          # local batches per core
SBLK = 512                # s-block
NSB = S // SBLK           # 4 s-blocks per batch
NCH = S // 128            # 16 s-chunks per batch
NJ = D // 128             # 8 contraction chunks
CPB = SBLK // 128         # 4 chunks per s-block
QX = Q + 1                # extra fused gate column/row
QXP = 128                 # padded stationary width: dual-fp8 LDWEIGHTS
                          # requires all 4 PE column groups active, so the
                          # stationary must span 128 columns (65.. are 0)

_prog_cache = {}


def _build(bi_v: float, cgate_v: float):
    nc = bacc.Bacc("TRN2", target_bir_lowering=False, debug=False,
                   enable_asserts=False, num_devices=NCORES)

    rawT_d = nc.dram_tensor("rawT", [BL, NJ, 128, S], F8,
                            kind="ExternalInput")
    rawN_d = nc.dram_tensor("rawN", [BL * NCH, 128, D], F8,
                            kind="ExternalInput")
    pdT_d = nc.dram_tensor("pdT", [BL, NJ, 128, S], F8,
                           kind="ExternalInput")
    pdN_d = nc.dram_tensor("pdN", [BL * NCH, 128, D], BF16,
                           kind="ExternalInput")
    qkx_d = nc.dram_tensor("qkx", [NJ, 128, QXP], F8, kind="ExternalInput")
    wox_d = nc.dram_tensor("wox", [NJ, 128, QXP], F8, kind="ExternalInput")
    boq_d = nc.dram_tensor("boq", [Q, 1], F32, kind="ExternalInput")
    wu2B_d = nc.dram_tensor("wu2B", [Q, D], F32, kind="ExternalInput")
    out_d = nc.dram_tensor("out", [BL * NCH, 128, D], BF16,
                           kind="ExternalOutput")

    with tile.TileContext(nc) as tc:
        with (
            tc.tile_pool(name="const", bufs=1) as cp,
            tc.tile_pool(name="dram", bufs=1, space="DRAM") as dram,
        ):
            # warm-up collective, dependency-free: the CC stream's
            # ~90us cold start (8-core barrier + first-collective setup)
            # runs during phase 1; contents are garbage and never read
            ar_w = dram.tile([1, 16], BF16)
            ar_wo = dram.tile([1, 16], BF16)
            nc.gpsimd.collective_compute(
                "AllReduce", ALU.add,
                replica_groups=[list(range(NCORES))],
                ins=[ar_w.opt()], outs=[ar_wo.opt()],
            )
            ident = cp.tile([128, 128], BF16, tag="ident")
            masks.make_identity(nc, ident[:])
            nbiB = cp.tile([128, 1], F32, tag="nbiB")
            nc.vector.memset(nbiB[:], -bi_v)
            ncgB = cp.tile([128, 1], F32, tag="ncgB")
            nc.vector.memset(ncgB[:], -cgate_v)

            qkx = cp.tile([128, NJ * QXP], F8, tag="qkx")
            wox = cp.tile([128, NJ * QXP], F8, tag="wox")
            boqc = cp.tile([Q, 1], F32, tag="boqc")
            wu2B = cp.tile([Q, D], F32, tag="wu2B")

            A_acc = cp.tile([Q, D], F32, tag="A_acc")
            A_bf = cp.tile([Q, D], BF16, tag="A_bf")
            awB = cp.tile([128, Q], BF16, tag="awB")
            scrA = cp.tile([Q, D], F32, tag="scrA")
            aw = cp.tile([Q, 1], F32, tag="aw")
            ar_in0 = dram.tile([Q + 1, D + 8], BF16)
            ar_out0 = dram.tile([Q + 1, D + 8], BF16)
            awz = cp.tile([Q, 8], BF16, tag="awz")
            nc.vector.memset(awz[:], 0.0)
            zrow = cp.tile([1, D + 8], BF16, tag="zrow")
            nc.vector.memset(zrow[:], 0.0)
            nc.gpsimd.dma_start(ar_in0[0:Q, D:D + 8], awz[:])
            nc.gpsimd.dma_start(ar_in0[Q:Q + 1, Q:D + 8],
                                zrow[:, Q:D + 8])

            # phase-2 pdT (both batches) + first pdN blocks prefetched
            # late in phase 1
            pdt0 = cp.tile([128, NJ * S], F8, tag="pdt0")
            pdt1 = cp.tile([128, NJ * S], F8, tag="pdt1")
            pdn_pre = [cp.tile([128, CPB * D], BF16, tag=f"pdnpre{k}",
                               name=f"pdnpre{k}")
                       for k in range(2)]

            def load_batchT(tile_, dram_t, b, s0=0, s1=S):
                nc.sync.dma_start(
                    tile_[:, :].rearrange("p (j s) -> p j s", j=NJ)
                    [:, :, s0:s1],
                    dram_t[b].rearrange("j p s -> p j s")[:, :, s0:s1])

            def load_n(tile_, dram_t, b, sb, eng):
                c0 = b * NCH + sb * CPB
                eng.dma_start(
                    tile_[:].rearrange("p (c d) -> p c d", c=CPB),
                    dram_t[c0:c0 + CPB].rearrange("c p d -> p c d"))

            # ================= PHASE 1 =================
            with (
                tc.tile_pool(name="p1", bufs=1) as p1,
                tc.tile_pool(name="p1ps", bufs=1, space="PSUM") as p1ps,
            ):
                def load_rn(b, sb):
                    t = p1.tile([128, CPB * D], F8, tag="rn",
                                name="rn", bufs=2)
                    load_n(t, rawN_d, b, sb, nc.scalar)
                    return t

                # weights first (tiny, needed by first matmuls)
                nc.sync.dma_start(
                    qkx[:].rearrange("p (j c) -> p j c", j=NJ),
                    qkx_d.rearrange("j p c -> p j c"))
                rawt = p1.tile([128, NJ * S], F8, tag="rawt0")
                load_batchT(rawt, rawT_d, 0, 0, SBLK)
                rn_cur = load_rn(0, 0)
                load_batchT(rawt, rawT_d, 0, SBLK, S)
                nc.sync.dma_start(
                    wox[:].rearrange("p (j c) -> p j c", j=NJ),
                    wox_d.rearrange("j p c -> p j c"))
                nc.sync.dma_start(boqc[:], boq_d[:])
                nc.sync.dma_start(wu2B[:], wu2B_d[:])

                for b in range(BL):
                    Zp = p1.tile([Q, NSB], F32, tag="Zp", bufs=2)
                    A_ps = p1ps.tile([128, D], F32, tag="A_ps", bufs=2)
                    for sb in range(NSB):
                        # prefetch next block's data
                        if sb + 1 < NSB:
                            rn_nxt = load_rn(b, sb + 1)
                        elif b + 1 < BL:
                            rn_nxt = load_rn(b + 1, 0)
                        else:
                            rn_nxt = None
                        if b == 0 and sb == 0:
                            rawt_nxt = p1.tile([128, NJ * S], F8,
                                               tag="rawt1")
                            load_batchT(rawt_nxt, rawT_d, 1)
                        if b == 1 and sb == 1:
                            load_batchT(pdt0, pdT_d, 0)
                            load_n(pdn_pre[0], pdN_d, 0, 0, nc.scalar)
                            load_n(pdn_pre[1], pdN_d, 0, 1, nc.scalar)
                        if b == 1 and sb == 2:
                            load_batchT(pdt1, pdT_d, 1)

                        # scores U[0:64] = exp(qk @ raw^T);
                        # row 64 = exp(-(raw.Wi + bi)) for the v-gate
                        sc_ps = p1ps.tile([QXP, SBLK], F32, tag="sc_ps",
                                          bufs=2)
                        qkx3 = qkx[:].rearrange("p (j c) -> p j c", j=NJ)
                        rawt3 = rawt[:].rearrange("p (j s) -> p j s", j=NJ)
                        for j in range(0, NJ, 2):
                            nc.tensor.matmul(
                                sc_ps[:], qkx3[:, j:j + 2, :],
                                rawt3[:, j:j + 2, sb * SBLK:
                                      (sb + 1) * SBLK],
                                start=(j == 0), stop=(j == NJ - 2),
                                perf_mode=DR)
                        U = p1.tile([QX, SBLK], BF16, tag="U", bufs=2)
                        nc.scalar.activation(U[0:Q, :], sc_ps[0:Q, :],
                                             AF.Exp, scale=1.0 / SCL,
                                             accum_out=Zp[:, sb:sb + 1])
                        nc.scalar.activation(U[Q:QX, :], sc_ps[Q:QX, :],
                                             AF.Exp, scale=-1.0 / SCL,
                                             bias=nbiB[0:1, :])
                        # transpose U chunks; fold g in on the way out
                        utp = None
                        rn3 = rn_cur[:].rearrange("p (c d) -> p c d",
                                                  c=CPB)
                        for cc in range(CPB):
                            ut_ps = p1ps.tile([128, QX], BF16, tag="ut_ps",
                                              bufs=2)
                            nc.tensor.transpose(
                                ut_ps[:], U[:, cc * 128:(cc + 1) * 128],
                                ident[:QX, :QX])
                            gcol = p1.tile([128, 1], F32, tag="gcol",
                                           bufs=4)
                            nc.vector.tensor_scalar_add(
                                gcol[:], ut_ps[:, Q:QX], 1.0)
                            nc.vector.reciprocal(gcol[:], gcol[:])
                            if cc % 2 == 0:
                                utp = p1.tile([128, 2 * 128], F8, tag="utp",
                                              name="utp", bufs=4)
                            nc.vector.tensor_scalar_mul(
                                utp[:, (cc % 2) * 128:
                                    (cc % 2) * 128 + Q],
                                ut_ps[:, 0:Q], gcol[:])
                            if cc % 2 == 1:
                                pr = sb * 2 + cc // 2
                                utp3 = utp[:].rearrange(
                                    "p (k m) -> p k m", k=2)
                                for h in range(2):
                                    nc.tensor.matmul(
                                        A_ps[:128, h * 512:(h + 1) * 512],
                                        utp3[:],
                                        rn3[:, cc - 1:cc + 1,
                                            h * 512:(h + 1) * 512],
                                        start=(pr == 0), stop=(pr == 7),
                                        skip_group_check=True,
                                        perf_mode=DR)
                        rn_cur = rn_nxt

                    # A_acc += A_ps / (16 * Z)
                    Z1 = p1.tile([Q, 1], F32, tag="Z1", bufs=2)
                    nc.vector.tensor_reduce(Z1[:], Zp[:], mybir.AxisListType.X,
                                            ALU.add)
                    sA = p1.tile([Q, 1], F32, tag="sA", bufs=2)
                    nc.vector.reciprocal(sA[:], Z1[:])
                    nc.vector.tensor_scalar_mul(sA[:], sA[:], 1.0 / B)
                    if b == 0:
                        nc.vector.tensor_scalar_mul(A_acc[:], A_ps[0:Q, :],
                                                    sA[:])
                        rawt = rawt_nxt
                    else:
                        nc.vector.scalar_tensor_tensor(
                            A_acc[:], A_ps[0:Q, :], sA[:], A_acc[:],
                            ALU.mult, ALU.add)

                # aw_local = A_acc . Wu2 rides along in AR0
                nc.vector.scalar_tensor_tensor(
                    scrA[:], A_acc[:], 1.0, wu2B[:],
                    ALU.mult, ALU.mult, accum_out=aw[:])
                nc.gpsimd.dma_start(ar_in0[0:Q, 0:D], A_acc[:])
                nc.gpsimd.dma_start(
                    ar_in0[Q:Q + 1, 0:Q].rearrange("a b -> b a"), aw[:])

            # ---- AllReduce of partial A (single collective: it is
            # latency-dominated, so one ~13us op beats two serial) ----
            nc.gpsimd.collective_compute(
                "AllReduce", ALU.add,
                replica_groups=[list(range(NCORES))],
                ins=[ar_in0.opt()], outs=[ar_out0.opt()],
            )
            nc.gpsimd.dma_start(A_bf[:], ar_out0[0:Q, 0:D])
            nc.gpsimd.dma_start(
                awB[:], ar_out0[Q:Q + 1, 0:Q].broadcast_to([128, Q]))

            # ================= PHASE 2 =================
            with (
                tc.tile_pool(name="p2", bufs=1) as p2,
                tc.tile_pool(name="p2ps", bufs=1, space="PSUM") as p2ps,
            ):
                def emit_partB(st):
                    (b, sb, Z2, G1, ut2x, u2s, pdn, SC) = st
                    G2 = p2.tile([128, CPB], F32, tag="G2", bufs=2)
                    for cc in range(CPB):
                        scr2 = p2.tile([128, Q], BF16, tag="scr2", bufs=2)
                        nc.vector.scalar_tensor_tensor(
                            scr2[:], u2s[cc][:], 1.0, awB[:],
                            ALU.mult, ALU.mult,
                            accum_out=G2[:, cc:cc + 1])
                    # SC = sigmoid(G1 + G2/Z2 + cg) / Z2  (exp-form)
                    rz = p2.tile([128, CPB], F32, tag="rz", bufs=2)
                    nc.vector.reciprocal(rz[:], Z2[:])
                    t4 = p2.tile([128, CPB], F32, tag="t4", bufs=2)
                    nc.vector.tensor_mul(t4[:], G2[:], rz[:])
                    nc.vector.tensor_add(t4[:], t4[:], G1[:])
                    e4 = p2.tile([128, CPB], F32, tag="e4", bufs=2)
                    nc.scalar.activation(e4[:], t4[:], AF.Exp,
                                         scale=-1.0, bias=ncgB[:])
                    nc.vector.tensor_scalar_add(e4[:], e4[:], 1.0)
                    nc.vector.reciprocal(e4[:], e4[:])
                    nc.vector.tensor_mul(SC[:], e4[:], rz[:])
                    emit_h(st, 0)
                    emit_h(st, 1)

                def emit_h(st, h):
                    (b, sb, Z2, G1, ut2x, u2s, pdn, SC) = st
                    oh = p2.tile([128, CPB * 512], BF16, tag=f"oh{h}",
                                 name=f"oh{h}", bufs=3)
                    for cc in range(CPB):
                        a2_ps = p2ps.tile([128, 512], F32, tag="a2_ps",
                                          bufs=3)
                        nc.tensor.matmul(
                            a2_ps[:],
                            ut2x[0:Q, cc * 128:(cc + 1) * 128],
                            A_bf[:, h * 512:(h + 1) * 512],
                            start=True, stop=True)
                        osl = slice(cc * 512, (cc + 1) * 512)
                        psl = slice(cc * D + h * 512, cc * D + (h + 1) * 512)
                        if h == 0:
                            nc.scalar.activation(oh[:, osl], a2_ps[:],
                                                 AF.Copy,
                                                 scale=SC[:, cc:cc + 1])
                            nc.vector.tensor_add(
                                oh[:, osl], oh[:, osl], pdn[:, psl])
                        elif cc < 2:
                            nc.vector.scalar_tensor_tensor(
                                oh[:, osl], a2_ps[:], SC[:, cc:cc + 1],
                                pdn[:, psl], ALU.mult, ALU.add)
                        else:
                            nc.scalar.activation(oh[:, osl], a2_ps[:],
                                                 AF.Copy,
                                                 scale=SC[:, cc:cc + 1])
                            nc.vector.tensor_add(
                                oh[:, osl], oh[:, osl], pdn[:, psl])
                    c0 = b * NCH + sb * CPB
                    nc.sync.dma_start(
                        out_d[c0:c0 + CPB, :, h * 512:(h + 1) * 512]
                        .rearrange("c p d -> p c d"),
                        oh[:].rearrange("p (c d) -> p c d", c=CPB))

                blocks = [(b, sb) for b in range(BL) for sb in range(NSB)]
                pdt_cur = pdt0
                pdn_queue = []
                pending = []
                for idx, (b, sb) in enumerate(blocks):
                    if b == 1 and sb == 0:
                        pdt_cur = pdt1
                    if idx + 2 < len(blocks):
                        nb, nsb2 = blocks[idx + 2]
                        pdn_n = p2.tile([128, CPB * D], BF16, tag="pdn",
                                        name="pdn", bufs=6)
                        load_n(pdn_n, pdN_d, nb, nsb2, nc.sync)
                        pdn_queue.append(pdn_n)
                    pdn = pdn_pre[idx] if idx < 2 else pdn_queue.pop(0)

                    # ---- partA: s2T + exp + transpose(+G1) ----
                    s2t_ps = p2ps.tile([QXP, SBLK], F32, tag="s2t_ps",
                                       bufs=2)
                    wox3 = wox[:].rearrange("p (j c) -> p j c", j=NJ)
                    pdt3 = pdt_cur[:].rearrange("p (j s) -> p j s", j=NJ)
                    for j in range(0, NJ, 2):
                        nc.tensor.matmul(
                            s2t_ps[:], wox3[:, j:j + 2, :],
                            pdt3[:, j:j + 2,
                                 sb * SBLK:(sb + 1) * SBLK],
                            start=(j == 0), stop=(j == NJ - 2),
                            perf_mode=DR)
                    ut2x = p2.tile([QX, SBLK], BF16, tag="ut2x", bufs=8)
                    nc.scalar.activation(ut2x[0:Q, :], s2t_ps[0:Q, :],
                                         AF.Exp, scale=1.0 / SCL,
                                         bias=boqc[:])
                    nc.scalar.activation(ut2x[Q:QX, :], s2t_ps[Q:QX, :],
                                         AF.Copy, scale=1.0 / SCL)
                    Z2 = p2.tile([128, CPB], F32, tag="Z2", bufs=8)
                    G1 = p2.tile([128, CPB], F32, tag="G1", bufs=8)
                    SC = p2.tile([128, CPB], F32, tag="SCp", name="SCp",
                                 bufs=8)
                    u2s = []
                    for cc in range(CPB):
                        u2c_ps = p2ps.tile([128, QX], BF16, tag="u2c_ps",
                                           bufs=2)
                        nc.tensor.transpose(
                            u2c_ps[:], ut2x[:, cc * 128:(cc + 1) * 128],
                            ident[:QX, :QX])
                        u2 = p2.tile([128, Q], BF16, tag=f"u2_{cc}",
                                     name=f"u2_{cc}", bufs=8)
                        nc.scalar.activation(u2[:], u2c_ps[:, 0:Q],
                                             AF.Copy,
                                             accum_out=Z2[:, cc:cc + 1])
                        nc.scalar.copy(G1[:, cc:cc + 1], u2c_ps[:, Q:QX])
                        u2s.append(u2)
                    pending.append((b, sb, Z2, G1, ut2x, u2s, pdn, SC))

                for st in pending:
                    emit_partB(st)

    nc.compile()
    return nc


def _get_prog(bi_v, cgate_v):
    key = (round(bi_v, 9), round(cgate_v, 9))
    if key not in _prog_cache:
        _prog_cache[key] = _build(bi_v, cgate_v)
    return _prog_cache[key]


def kernel(raw, post_dec, mask, questions, Wk, bk, Wi, bi, Wo, bo,
           Wu1, bu1, Wu2, bu2, b1, _trace=False):
    raw = np.asarray(raw, dtype=np.float32)
    post_dec = np.asarray(post_dec, dtype=np.float32)
    questions = np.asarray(questions, dtype=np.float32)
    Wk = np.asarray(Wk, dtype=np.float32)
    Wo = np.asarray(Wo, dtype=np.float32)

    bi_v = float(np.asarray(bi).reshape(-1)[0])
    cgate_v = float(np.asarray(bu1).reshape(-1)[0]
                    + np.asarray(bu2).reshape(-1)[0]
                    + np.asarray(b1).reshape(-1)[0])
    nc = _get_prog(bi_v, cgate_v)

    inv_sqrt_d = np.float32(1.0 / np.sqrt(D))
    inv_sqrt_q = np.float32(1.0 / np.sqrt(Q))
    # stationaries with the fused gate column
    qkx = np.zeros((D, QXP), np.float32)
    qkx[:, 0:Q] = (questions @ Wk).T * inv_sqrt_d
    qkx[:, Q] = np.asarray(Wi, np.float32).reshape(D)
    wox = np.zeros((D, QXP), np.float32)
    wox[:, 0:Q] = (questions @ Wo).T * inv_sqrt_q
    wox[:, Q] = np.asarray(Wu1, np.float32).reshape(D)
    qkx = np.ascontiguousarray(qkx.reshape(NJ, 128, QXP) * SCL).astype(F8NP)
    wox = np.ascontiguousarray(wox.reshape(NJ, 128, QXP) * SCL).astype(F8NP)
    boq = np.ascontiguousarray(
        ((questions @ np.asarray(bo, np.float32)) * inv_sqrt_q
         ).reshape(Q, 1)).astype(np.float32)
    wu2B = np.ascontiguousarray(
        np.broadcast_to(np.asarray(Wu2, np.float32).reshape(1, D), (Q, D)))

    in_maps = []
    for r in range(NCORES):
        bs = slice(r * BL, (r + 1) * BL)
        rawT = np.ascontiguousarray(
            raw[bs].transpose(0, 2, 1)).astype(F8NP).reshape(
            BL, NJ, 128, S)
        rawN = np.ascontiguousarray(raw[bs]).astype(F8NP).reshape(
            BL * NCH, 128, D)
        pdT = np.ascontiguousarray(
            post_dec[bs].transpose(0, 2, 1)).astype(F8NP).reshape(
            BL, NJ, 128, S)
        pdN = np.ascontiguousarray(post_dec[bs]).astype(BF).reshape(
            BL * NCH, 128, D)
        in_maps.append({
            "rawT": rawT, "rawN": rawN, "pdT": pdT, "pdN": pdN,
            "qkx": qkx, "wox": wox, "boq": boq, "wu2B": wu2B,
        })

    res = run_bass_kernel_spmd(nc, in_maps, core_ids=list(range(NCORES)),
                               trace=_trace)
    out = np.concatenate(
        [res.results[r]["out"].astype(np.float32).reshape(BL, S, D)
         for r in range(NCORES)],
        axis=0)
    if _trace:
        kernel._last_result = res
    return out


# revision 29
# speedup vs baseline: 1.2969x; 1.0649x over previous
"""Trainium2 Bass kernel for nn_Pndb_43344809951805 (scatter_memory).

Data-parallel over batch B=16 across 8 NeuronCores (2 batches/core).

Algebraic rewrites vs the reference:
  Phase 1: scores = (questions @ Wk) @ raw^T  (q.bk bias is softmax-
           invariant over s). Wi is folded in as a 65th stationary
           column, so the v-gate logit row comes free with the scores
           matmul; sigma(g) rides the U transpose and scales the attn
           rows per-partition.
  Phase 2: one [65,512] matmul group per block (stationary = woq chunk
           plus a Wu1 column) yields the read logits transposed and the
           G1 gate row. boq enters as the exp activation's
           per-partition bias.
Scalar engine runs Exp/Copy only (sigmoids via exp to avoid activation
table reloads). DMA is split across both HWDGE rings (sync=SP,
scalar=ACT). HBM-bound: ~42 MB/core.
"""
import sys

sys.path.insert(0, "/opt/trn_rl_repo")

import numpy as np
import ml_dtypes

import concourse.bass as bass
import concourse.bacc as bacc
import concourse.mybir as mybir
import concourse.tile as tile
from concourse import masks
from concourse.bass_utils import run_bass_kernel_spmd

F32 = mybir.dt.float32
BF16 = mybir.dt.bfloat16
F8 = mybir.dt.float8e4
SCL = 64.0
AF = mybir.ActivationFunctionType
ALU = mybir.AluOpType
BF = ml_dtypes.bfloat16
F8NP = ml_dtypes.float8_e4m3fn
DR = mybir.MatmulPerfMode.DoubleRow

B, S, D, Q = 16, 2048, 1024, 64
NCORES = 8
BL = B // NCORES          # local batches per core
SBLK = 512                # s-block
NSB = S // SBLK           # 4 s-blocks per batch
NCH = S // 128            # 16 s-chunks per batch
NJ = D // 128             # 8 contraction chunks
CPB = SBLK // 128         # 4 chunks per s-block
QX = Q + 1                # extra fused gate column/row
QXP = 128                 # padded stationary width: dual-fp8 LDWEIGHTS
                          # requires all 4 PE column groups active, so the
                          # stationary must span 128 columns (65.. are 0)

_prog_cache = {}


def _build(bi_v: float, cgate_v: float):
    nc = bacc.Bacc("TRN2", target_bir_lowering=False, debug=False,
                   enable_asserts=False, num_devices=NCORES)

    rawT_d = nc.dram_tensor("rawT", [BL, NJ, 128, S], F8,
                            kind="ExternalInput")
    rawN_d = nc.dram_tensor("rawN", [BL * NCH, 128, D], F8,
                            kind="ExternalInput")
    pdT_d = nc.dram_tensor("pdT", [BL, NJ, 128, S], F8,
                           kind="ExternalInput")
    pdN_d = nc.dram_tensor("pdN", [BL * NCH, 128, D], BF16,
                           kind="ExternalInput")
    qkx_d = nc.dram_tensor("qkx", [NJ, 128, QXP], F8, kind="ExternalInput")
    wox_d = nc.dram_tensor("wox", [NJ, 128, QXP], F8, kind="ExternalInput")
    boq_d = nc.dram_tensor("boq", [Q, 1], F32, kind="ExternalInput")
    wu2B_d = nc.dram_tensor("wu2B", [Q, D], F32, kind="ExternalInput")
    out_d = nc.dram_tensor("out", [BL * NCH, 128, D], BF16,
                           kind="ExternalOutput")

    with tile.TileContext(nc) as tc:
        with (
            tc.tile_pool(name="const", bufs=1) as cp,
            tc.tile_pool(name="dram", bufs=1, space="DRAM") as dram,
        ):
            # warm-up collective, dependency-free: the CC stream's
            # ~90us cold start (8-core barrier + first-collective setup)
            # runs during phase 1; contents are garbage and never read
            ar_w = dram.tile([1, 16], BF16)
            ar_wo = dram.tile([1, 16], BF16)
            nc.gpsimd.collective_compute(
                "AllReduce", ALU.add,
                replica_groups=[list(range(NCORES))],
                ins=[ar_w.opt()], outs=[ar_wo.opt()],
            )
            ident = cp.tile([128, 128], BF16, tag="ident")
            masks.make_identity(nc, ident[:])
            nbiB = cp.tile([128, 1], F32, tag="nbiB")
            nc.vector.memset(nbiB[:], -bi_v)
            ncgB = cp.tile([128, 1], F32, tag="ncgB")
            nc.vector.memset(ncgB[:], -cgate_v)

            qkx = cp.tile([128, NJ * QXP], F8, tag="qkx")
            wox = cp.tile([128, NJ * QXP], F8, tag="wox")
            boqc = cp.tile([Q, 1], F32, tag="boqc")
            wu2B = cp.tile([Q, D], F32, tag="wu2B")

            A_acc = cp.tile([Q, D], F32, tag="A_acc")
            A_bf = cp.tile([Q, D], BF16, tag="A_bf")
            awB = cp.tile([128, Q], BF16, tag="awB")
            scrA = cp.tile([Q, D], F32, tag="scrA")
            aw = cp.tile([Q, 1], F32, tag="aw")
            ar_in0 = dram.tile([Q + 1, D + 8], BF16)
            ar_out0 = dram.tile([Q + 1, D + 8], BF16)
            awz = cp.tile([Q, 8], BF16, tag="awz")
            nc.vector.memset(awz[:], 0.0)
            zrow = cp.tile([1, D + 8], BF16, tag="zrow")
            nc.vector.memset(zrow[:], 0.0)
            nc.gpsimd.dma_start(ar_in0[0:Q, D:D + 8], awz[:])
            nc.gpsimd.dma_start(ar_in0[Q:Q + 1, Q:D + 8],
                                zrow[:, Q:D + 8])

            # phase-2 pdT (both batches) + first pdN blocks prefetched
            # late in phase 1
            pdt0 = cp.tile([128, NJ * S], F8, tag="pdt0")
            pdt1 = cp.tile([128, NJ * S], F8, tag="pdt1")
            pdn_pre = [cp.tile([128, CPB * D], BF16, tag=f"pdnpre{k}",
                               name=f"pdnpre{k}")
                       for k in range(2)]

            def load_batchT(tile_, dram_t, b, s0=0, s1=S):
                nc.sync.dma_start(
                    tile_[:, :].rearrange("p (j s) -> p j s", j=NJ)
                    [:, :, s0:s1],
                    dram_t[b].rearrange("j p s -> p j s")[:, :, s0:s1])

            def load_n(tile_, dram_t, b, sb, eng):
                c0 = b * NCH + sb * CPB
                eng.dma_start(
                    tile_[:].rearrange("p (c d) -> p c d", c=CPB),
                    dram_t[c0:c0 + CPB].rearrange("c p d -> p c d"))

            # ================= PHASE 1 =================
            with (
                tc.tile_pool(name="p1", bufs=1) as p1,
                tc.tile_pool(name="p1ps", bufs=1, space="PSUM") as p1ps,
            ):
                def load_rn(b, sb):
                    t = p1.tile([128, CPB * D], F8, tag="rn",
                                name="rn", bufs=2)
                    load_n(t, rawN_d, b, sb, nc.scalar)
                    return t

                # weights first (tiny, needed by first matmuls)
                nc.sync.dma_start(
                    qkx[:].rearrange("p (j c) -> p j c", j=NJ),
                    qkx_d.rearrange("j p c -> p j c"))
                rawt = p1.tile([128, NJ * S], F8, tag="rawt0")
                load_batchT(rawt, rawT_d, 0, 0, SBLK)
                rn_cur = load_rn(0, 0)
                load_batchT(rawt, rawT_d, 0, SBLK, S)
                nc.sync.dma_start(
                    wox[:].rearrange("p (j c) -> p j c", j=NJ),
                    wox_d.rearrange("j p c -> p j c"))
                nc.sync.dma_start(boqc[:], boq_d[:])
                nc.sync.dma_start(wu2B[:], wu2B_d[:])

                for b in range(BL):
                    Zp = p1.tile([Q, NSB], F32, tag="Zp", bufs=2)
                    A_ps = p1ps.tile([128, D], F32, tag="A_ps", bufs=2)
                    for sb in range(NSB):
                        # prefetch next block's data
                        if sb + 1 < NSB:
                            rn_nxt = load_rn(b, sb + 1)
                        elif b + 1 < BL:
                            rn_nxt = load_rn(b + 1, 0)
                        else:
                            rn_nxt = None
                        if b == 0 and sb == 0:
                            rawt_nxt = p1.tile([128, NJ * S], F8,
                                               tag="rawt1")
                            load_batchT(rawt_nxt, rawT_d, 1)
                        if b == 1 and sb == 1:
                            load_batchT(pdt0, pdT_d, 0)
                            load_n(pdn_pre[0], pdN_d, 0, 0, nc.scalar)
                            load_n(pdn_pre[1], pdN_d, 0, 1, nc.scalar)

                        # scores U[0:64] = exp(qk @ raw^T);
                        # row 64 = exp(-(raw.Wi + bi)) for the v-gate
                        sc_ps = p1ps.tile([QXP, SBLK], F32, tag="sc_ps",
                                          bufs=2)
                        qkx3 = qkx[:].rearrange("p (j c) -> p j c", j=NJ)
                        rawt3 = rawt[:].rearrange("p (j s) -> p j s", j=NJ)
                        for j in range(0, NJ, 2):
                            nc.tensor.matmul(
                                sc_ps[:], qkx3[:, j:j + 2, :],
                                rawt3[:, j:j + 2, sb * SBLK:
                                      (sb + 1) * SBLK],
                                start=(j == 0), stop=(j == NJ - 2),
                                perf_mode=DR)
                        U = p1.tile([QX, SBLK], BF16, tag="U", bufs=2)
                        nc.scalar.activation(U[0:Q, :], sc_ps[0:Q, :],
                                             AF.Exp, scale=1.0 / SCL,
                                             accum_out=Zp[:, sb:sb + 1])
                        nc.scalar.activation(U[Q:QX, :], sc_ps[Q:QX, :],
                                             AF.Exp, scale=-1.0 / SCL,
                                             bias=nbiB[0:1, :])
                        # transpose U chunks; fold g in on the way out
                        utp = None
                        rn3 = rn_cur[:].rearrange("p (c d) -> p c d",
                                                  c=CPB)
                        for cc in range(CPB):
                            ut_ps = p1ps.tile([128, QX], BF16, tag="ut_ps",
                                              bufs=2)
                            nc.tensor.transpose(
                                ut_ps[:], U[:, cc * 128:(cc + 1) * 128],
                                ident[:QX, :QX])
                            gcol = p1.tile([128, 1], F32, tag="gcol",
                                           bufs=4)
                            nc.vector.tensor_scalar_add(
                                gcol[:], ut_ps[:, Q:QX], 1.0)
                            nc.vector.reciprocal(gcol[:], gcol[:])
                            if cc % 2 == 0:
                                utp = p1.tile([128, 2 * 128], F8, tag="utp",
                                              name="utp", bufs=4)
                            nc.vector.tensor_scalar_mul(
                                utp[:, (cc % 2) * 128:
                                    (cc % 2) * 128 + Q],
                                ut_ps[:, 0:Q], gcol[:])
                            if cc % 2 == 1:
                                pr = sb * 2 + cc // 2
                                utp3 = utp[:].rearrange(
                                    "p (k m) -> p k m", k=2)
                                for h in range(2):
                                    nc.tensor.matmul(
                                        A_ps[:128, h * 512:(h + 1) * 512],
                                        utp3[:],
                                        rn3[:, cc - 1:cc + 1,
                                            h * 512:(h + 1) * 512],
                                        start=(pr == 0), stop=(pr == 7),
                                        skip_group_check=True,
                                        perf_mode=DR)
                        rn_cur = rn_nxt

                    # A_acc += A_ps / (16 * Z)
                    Z1 = p1.tile([Q, 1], F32, tag="Z1", bufs=2)
                    nc.vector.tensor_reduce(Z1[:], Zp[:], mybir.AxisListType.X,
                                            ALU.add)
                    sA = p1.tile([Q, 1], F32, tag="sA", bufs=2)
                    nc.vector.reciprocal(sA[:], Z1[:])
                    nc.vector.tensor_scalar_mul(sA[:], sA[:], 1.0 / B)
                    if b == 0:
                        nc.vector.tensor_scalar_mul(A_acc[:], A_ps[0:Q, :],
                                                    sA[:])
                        rawt = rawt_nxt
                    else:
                        nc.vector.scalar_tensor_tensor(
                            A_acc[:], A_ps[0:Q, :], sA[:], A_acc[:],
                            ALU.mult, ALU.add)

                # aw_local = A_acc . Wu2 rides along in AR0
                nc.vector.scalar_tensor_tensor(
                    scrA[:], A_acc[:], 1.0, wu2B[:],
                    ALU.mult, ALU.mult, accum_out=aw[:])
                nc.gpsimd.dma_start(ar_in0[0:Q, 0:D], A_acc[:])
                nc.gpsimd.dma_start(
                    ar_in0[Q:Q + 1, 0:Q].rearrange("a b -> b a"), aw[:])
                # pdt1 lands late on purpose: partA blocks 4-7 then run
                # inside the AllReduce window, keeping the PE warm
                load_batchT(pdt1, pdT_d, 1)

            # ---- AllReduce of partial A (single collective: it is
            # latency-dominated, so one ~13us op beats two serial) ----
            nc.gpsimd.collective_compute(
                "AllReduce", ALU.add,
                replica_groups=[list(range(NCORES))],
                ins=[ar_in0.opt()], outs=[ar_out0.opt()],
            )
            nc.gpsimd.dma_start(A_bf[:], ar_out0[0:Q, 0:D])
            nc.gpsimd.dma_start(
                awB[:], ar_out0[Q:Q + 1, 0:Q].broadcast_to([128, Q]))

            # ================= PHASE 2 =================
            with (
                tc.tile_pool(name="p2", bufs=1) as p2,
                tc.tile_pool(name="p2ps", bufs=1, space="PSUM") as p2ps,
            ):
                def emit_partB(st):
                    (b, sb, Z2, G1, ut2x, u2s, pdn, SC) = st
                    G2 = p2.tile([128, CPB], F32, tag="G2", bufs=2)
                    for cc in range(CPB):
                        scr2 = p2.tile([128, Q], BF16, tag="scr2", bufs=2)
                        nc.vector.scalar_tensor_tensor(
                            scr2[:], u2s[cc][:], 1.0, awB[:],
                            ALU.mult, ALU.mult,
                            accum_out=G2[:, cc:cc + 1])
                    # SC = sigmoid(G1 + G2/Z2 + cg) / Z2  (exp-form)
                    rz = p2.tile([128, CPB], F32, tag="rz", bufs=2)
                    nc.vector.reciprocal(rz[:], Z2[:])
                    t4 = p2.tile([128, CPB], F32, tag="t4", bufs=2)
                    nc.vector.tensor_mul(t4[:], G2[:], rz[:])
                    nc.vector.tensor_add(t4[:], t4[:], G1[:])
                    e4 = p2.tile([128, CPB], F32, tag="e4", bufs=2)
                    nc.scalar.activation(e4[:], t4[:], AF.Exp,
                                         scale=-1.0, bias=ncgB[:])
                    nc.vector.tensor_scalar_add(e4[:], e4[:], 1.0)
                    nc.vector.reciprocal(e4[:], e4[:])
                    nc.vector.tensor_mul(SC[:], e4[:], rz[:])
                    emit_h(st, 0)
                    emit_h(st, 1)

                def emit_h(st, h):
                    (b, sb, Z2, G1, ut2x, u2s, pdn, SC) = st
                    oh = p2.tile([128, CPB * 512], BF16, tag=f"oh{h}",
                                 name=f"oh{h}", bufs=3)
                    for cc in range(CPB):
                        a2_ps = p2ps.tile([128, 512], F32, tag="a2_ps",
                                          bufs=3)
                        nc.tensor.matmul(
                            a2_ps[:],
                            ut2x[0:Q, cc * 128:(cc + 1) * 128],
                            A_bf[:, h * 512:(h + 1) * 512],
                            start=True, stop=True)
                        osl = slice(cc * 512, (cc + 1) * 512)
                        psl = slice(cc * D + h * 512, cc * D + (h + 1) * 512)
                        if h == 0:
                            nc.scalar.activation(oh[:, osl], a2_ps[:],
                                                 AF.Copy,
                                                 scale=SC[:, cc:cc + 1])
                            nc.vector.tensor_add(
                                oh[:, osl], oh[:, osl], pdn[:, psl])
                        elif cc < 2:
                            nc.vector.scalar_tensor_tensor(
                                oh[:, osl], a2_ps[:], SC[:, cc:cc + 1],
                                pdn[:, psl], ALU.mult, ALU.add)
                        else:
                            nc.scalar.activation(oh[:, osl], a2_ps[:],
                                                 AF.Copy,
                                                 scale=SC[:, cc:cc + 1])
                            nc.vector.tensor_add(
                                oh[:, osl], oh[:, osl], pdn[:, psl])
                    c0 = b * NCH + sb * CPB
                    nc.sync.dma_start(
                        out_d[c0:c0 + CPB, :, h * 512:(h + 1) * 512]
                        .rearrange("c p d -> p c d"),
                        oh[:].rearrange("p (c d) -> p c d", c=CPB))

                blocks = [(b, sb) for b in range(BL) for sb in range(NSB)]
                pdt_cur = pdt0
                pdn_queue = []
                pending = []
                for idx, (b, sb) in enumerate(blocks):
                    if b == 1 and sb == 0:
                        pdt_cur = pdt1
                    if idx + 2 < len(blocks):
                        nb, nsb2 = blocks[idx + 2]
                        pdn_n = p2.tile([128, CPB * D], BF16, tag="pdn",
                                        name="pdn", bufs=6)
                        load_n(pdn_n, pdN_d, nb, nsb2, nc.sync)
                        pdn_queue.append(pdn_n)
                    pdn = pdn_pre[idx] if idx < 2 else pdn_queue.pop(0)

                    # ---- partA: s2T + exp + transpose(+G1) ----
                    s2t_ps = p2ps.tile([QXP, SBLK], F32, tag="s2t_ps",
                                       bufs=2)
                    wox3 = wox[:].rearrange("p (j c) -> p j c", j=NJ)
                    pdt3 = pdt_cur[:].rearrange("p (j s) -> p j s", j=NJ)
                    for j in range(0, NJ, 2):
                        nc.tensor.matmul(
                            s2t_ps[:], wox3[:, j:j + 2, :],
                            pdt3[:, j:j + 2,
                                 sb * SBLK:(sb + 1) * SBLK],
                            start=(j == 0), stop=(j == NJ - 2),
                            perf_mode=DR)
                    ut2x = p2.tile([QX, SBLK], BF16, tag="ut2x", bufs=8)
                    nc.scalar.activation(ut2x[0:Q, :], s2t_ps[0:Q, :],
                                         AF.Exp, scale=1.0 / SCL,
                                         bias=boqc[:])
                    nc.scalar.activation(ut2x[Q:QX, :], s2t_ps[Q:QX, :],
                                         AF.Copy, scale=1.0 / SCL)
                    Z2 = p2.tile([128, CPB], F32, tag="Z2", bufs=8)
                    G1 = p2.tile([128, CPB], F32, tag="G1", bufs=8)
                    SC = p2.tile([128, CPB], F32, tag="SCp", name="SCp",
                                 bufs=8)
                    u2s = []
                    for cc in range(CPB):
                        u2c_ps = p2ps.tile([128, QX], BF16, tag="u2c_ps",
                                           bufs=2)
                        nc.tensor.transpose(
                            u2c_ps[:], ut2x[:, cc * 128:(cc + 1) * 128],
                            ident[:QX, :QX])
                        u2 = p2.tile([128, Q], BF16, tag=f"u2_{cc}",
                                     name=f"u2_{cc}", bufs=8)
                        nc.scalar.activation(u2[:], u2c_ps[:, 0:Q],
                                             AF.Copy,
                                             accum_out=Z2[:, cc:cc + 1])
                        nc.scalar.copy(G1[:, cc:cc + 1], u2c_ps[:, Q:QX])
                        u2s.append(u2)
                    pending.append((b, sb, Z2, G1, ut2x, u2s, pdn, SC))

                for st in pending:
                    emit_partB(st)

    nc.compile()
    return nc


def _get_prog(bi_v, cgate_v):
    key = (round(bi_v, 9), round(cgate_v, 9))
    if key not in _prog_cache:
        _prog_cache[key] = _build(bi_v, cgate_v)
    return _prog_cache[key]


def kernel(raw, post_dec, mask, questions, Wk, bk, Wi, bi, Wo, bo,
           Wu1, bu1, Wu2, bu2, b1, _trace=False):
    raw = np.asarray(raw, dtype=np.float32)
    post_dec = np.asarray(post_dec, dtype=np.float32)
    questions = np.asarray(questions, dtype=np.float32)
    Wk = np.asarray(Wk, dtype=np.float32)
    Wo = np.asarray(Wo, dtype=np.float32)

    bi_v = float(np.asarray(bi).reshape(-1)[0])
    cgate_v = float(np.asarray(bu1).reshape(-1)[0]
                    + np.asarray(bu2).reshape(-1)[0]
                    + np.asarray(b1).reshape(-1)[0])
    nc = _get_prog(bi_v, cgate_v)

    inv_sqrt_d = np.float32(1.0 / np.sqrt(D))
    inv_sqrt_q = np.float32(1.0 / np.sqrt(Q))
    # stationaries with the fused gate column
    qkx = np.zeros((D, QXP), np.float32)
    qkx[:, 0:Q] = (questions @ Wk).T * inv_sqrt_d
    qkx[:, Q] = np.asarray(Wi, np.float32).reshape(D)
    wox = np.zeros((D, QXP), np.float32)
    wox[:, 0:Q] = (questions @ Wo).T * inv_sqrt_q
    wox[:, Q] = np.asarray(Wu1, np.float32).reshape(D)
    qkx = np.ascontiguousarray(qkx.reshape(NJ, 128, QXP) * SCL).astype(F8NP)
    wox = np.ascontiguousarray(wox.reshape(NJ, 128, QXP) * SCL).astype(F8NP)
    boq = np.ascontiguousarray(
        ((questions @ np.asarray(bo, np.float32)) * inv_sqrt_q
         ).reshape(Q, 1)).astype(np.float32)
    wu2B = np.ascontiguousarray(
        np.broadcast_to(np.asarray(Wu2, np.float32).reshape(1, D), (Q, D)))

    in_maps = []
    for r in range(NCORES):
        bs = slice(r * BL, (r + 1) * BL)
        rawT = np.ascontiguousarray(
            raw[bs].transpose(0, 2, 1)).astype(F8NP).reshape(
            BL, NJ, 128, S)
        rawN = np.ascontiguousarray(raw[bs]).astype(F8NP).reshape(
            BL * NCH, 128, D)
        pdT = np.ascontiguousarray(
            post_dec[bs].transpose(0, 2, 1)).astype(F8NP).reshape(
            BL, NJ, 128, S)
        pdN = np.ascontiguousarray(post_dec[bs]).astype(BF).reshape(
            BL * NCH, 128, D)
        in_maps.append({
            "rawT": rawT, "rawN": rawN, "pdT": pdT, "pdN": pdN,
            "qkx": qkx, "wox": wox, "boq": boq, "wu2B": wu2B,
        })

    res = run_bass_kernel_spmd(nc, in_maps, core_ids=list(range(NCORES)),
                               trace=_trace)
    out = np.concatenate(
        [res.results[r]["out"].astype(np.float32).reshape(BL, S, D)
         for r in range(NCORES)],
        axis=0)
    if _trace:
        kernel._last_result = res
    return out
